# revision 17
# baseline (speedup 1.0000x reference)
"""Trainium2 Bass kernel for nn_Attention_386547057357 (Transformer-XL style
relative-position sparse attention).

Sharding: data-parallel over batch — core c computes batch element c.
All weights replicated per core.

Math (per batch element):
    X = [memory; x]  (1024, 512)
    q = x @ W_q  (256, 512);  k = X @ W_k;  v = X @ W_v
    qhat = q + u_emb (per head);  qtld = q + v_emb
    RW = R @ W_rel  (1024, 512)     [host-precomputed constant]
    ac[n, m]  = qhat_h[n] . k_h[m]            (= term_a + term_c)
    bd[n, r'] = qtld_h[n] . RW[1023 - r']     (= term_b + term_d, reversed r)
    scores[n, m] = (ac[n, m] + bd[n, 255 - n + m]) * scale  + causal mask
    out = softmax(scores) @ v @ W_out + b_out

Implementation notes (ADD-form):
  - the relative shift is a DRAM round trip in bf16: write RAW bd rows with
    stride 1281 (cols [1024:1281] = -inf pad -> exp gives 0, the causal
    mask), read back diagonally [[1280, 128], [1, 1024]] at offset 255 with
    DMA accum_op=add onto a tile already holding RAW ac.
  - single ACT exp (scale) bf16-out with fused accum_out rowsum.
  - normalize = DVE tensor_scalar (bf16 tensor x f32 per-partition scalar).
  - attn transpose for the PV matmul via SBUF-source dma_gather(transpose).
  - all PE operands bf16; PSUM accumulation fp32; output fp32.
"""

import sys

sys.path.insert(0, "/opt/trn_rl_repo")

import numpy as np
import ml_dtypes

import concourse.bass as bass
import concourse.mybir as mybir
import concourse.tile as tile
from concourse import bacc, library_config
from concourse.bass_utils import run_bass_kernel_spmd
from concourse.tile_rust import add_dep_helper

BF16 = ml_dtypes.bfloat16
F32 = np.float32

DIM = 512
NHEAD = 8
DHEAD = 64
CTX = 1024
NOCT = 11
B = 8
SEQ = 256
MEM = 768
TOT = MEM + SEQ  # 1024
SCALE = DHEAD ** -0.5  # 0.125
RSTRIDE = 1281  # bd scratch row stride (1024 data + 257 pad)
PADVAL = -120.0  # pad logit: exp(SCALE * (ac - 120)) ~ e^-15 ~ 0 (causal mask)

dt = mybir.dt
AF = mybir.ActivationFunctionType
ALU = mybir.AluOpType


# ---------------------------------------------------------------- host consts
def _positional_encoding():
    coords = np.arange(CTX, dtype=F32)[:, None]
    octaves = np.arange(1 - NOCT, 1, dtype=F32)
    mult = ((2.0 ** octaves) * np.pi).astype(F32)
    scaled = (coords * mult[None, :]).astype(F32)
    return np.concatenate([np.sin(scaled), np.cos(scaled)], axis=-1).astype(F32)


def _chunked(w, nchunk):
    """(128*nchunk, F) -> (128, nchunk, F) with [p, c, f] = w[128c + p, f]."""
    f = w.shape[1]
    return np.ascontiguousarray(w.reshape(nchunk, 128, f).transpose(1, 0, 2))


# ---------------------------------------------------------------- bass program
def build_program():
    nc = bacc.Bacc("TRN2", target_bir_lowering=False, debug=False)

    xt_d = nc.dram_tensor("xt", [128, 4, TOT], dt.bfloat16, kind="ExternalInput")
    wqkv_d = nc.dram_tensor("wqkv", [128, 4, 1536], dt.bfloat16, kind="ExternalInput")
    rwt_d = nc.dram_tensor("rwt", [128, 4, CTX], dt.bfloat16, kind="ExternalInput")
    wout_d = nc.dram_tensor("wout", [128, 4, 512], dt.bfloat16, kind="ExternalInput")
    bout_d = nc.dram_tensor("bout", [128, 512], dt.float32, kind="ExternalInput")
    u2_d = nc.dram_tensor("u2", [128, 1], dt.float32, kind="ExternalInput")
    v2_d = nc.dram_tensor("v2", [128, 1], dt.float32, kind="ExternalInput")
    gidx_d = nc.dram_tensor("gidx", [128, 16], dt.int16, kind="ExternalInput")
    out_d = nc.dram_tensor("out", [SEQ, 512], dt.float32, kind="ExternalOutput")

    with tile.TileContext(nc) as tc:
        _body(tc, xt_d, wqkv_d, rwt_d, wout_d, bout_d, u2_d, v2_d,
              gidx_d, out_d)
    nc.compile()
    return nc


def _body(tc, xt_d, wqkv_d, rwt_d, wout_d, bout_d, u2_d, v2_d, gidx_d, out_d):
    nc = tc.nc
    from contextlib import ExitStack

    with ExitStack() as ctx:
        consts = ctx.enter_context(tc.tile_pool(name="consts", bufs=1))

        # ---- load constants / weights.  xt+wqkv split per chunk so the
        # first projection matmuls can start before the tail of the loads.
        xt = consts.tile([128, 4, TOT], dt.bfloat16)
        wqkv = consts.tile([128, 4, 1536], dt.bfloat16)
        for ch in range(2):
            nc.sync.dma_start(xt[:, ch, :], xt_d.ap()[:, ch, :])
            nc.sync.dma_start(wqkv[:, ch, :], wqkv_d.ap()[:, ch, :])
        u2 = consts.tile([128, 1], dt.float32)
        nc.sync.dma_start(u2[:], u2_d.ap())
        v2 = consts.tile([128, 1], dt.float32)
        nc.sync.dma_start(v2[:], v2_d.ap())
        gidx = consts.tile([128, 16], dt.int16)
        nc.sync.dma_start(gidx[:], gidx_d.ap())
        for ch in range(2, 4):
            nc.sync.dma_start(xt[:, ch, :], xt_d.ap()[:, ch, :])
            nc.sync.dma_start(wqkv[:, ch, :], wqkv_d.ap()[:, ch, :])
        rwt = consts.tile([128, 4, CTX], dt.bfloat16)
        nc.sync.dma_start(rwt[:], rwt_d.ap())
        wout = consts.tile([128, 4, 512], dt.bfloat16)
        nc.sync.dma_start(wout[:], wout_d.ap())
        bout = consts.tile([128, 512], dt.float32)
        nc.sync.dma_start(bout[:], bout_d.ap())

        # persistent intermediates
        qhatT = consts.tile([128, 4, SEQ], dt.bfloat16)  # (q+u)^T  [hd, n]
        qtldT = consts.tile([128, 4, SEQ], dt.bfloat16)  # (q+v)^T  [hd, n]
        kT = consts.tile([128, 4, TOT], dt.bfloat16)     # k^T      [hd, m]
        vv = consts.tile([128, 8, 512], dt.bfloat16)     # V        [m, hd]
        avt = consts.tile([128, 4, SEQ], dt.bfloat16)    # attnV^T  [hd, n]

        with (
            tc.tile_pool(name="mps", bufs=3, space="PSUM") as mps,
            tc.tile_pool(name="pvps", bufs=2, space="PSUM") as pvps,
            tc.tile_pool(name="hsb", bufs=4) as hsb,
            tc.tile_pool(name="ebp", bufs=6) as ebp,
            tc.tile_pool(name="bdd", bufs=6, space="DRAM") as bddp,
        ):
            # alternate PSUM->SBUF cast copies between ACT and DVE to
            # balance the two engines
            _cp = [0]

            def copy_ps(dst, src):
                _cp[0] ^= 1
                if _cp[0]:
                    nc.scalar.copy(dst, src)
                else:
                    nc.vector.tensor_copy(dst, src)

            # q^T per head pair, then add u/v
            for hp in range(4):
                psw = mps.tile([128, 1024], dt.float32, tag="m")
                ps = psw[:, 0:SEQ]
                for ch in range(4):
                    nc.tensor.matmul(ps, wqkv[:, ch, 128 * hp:128 * (hp + 1)],
                                     xt[:, ch, MEM:TOT],
                                     start=(ch == 0), stop=(ch == 3))
                # tensor_scalar with f32 psum in, bf16 out (probe-verified)
                nc.vector.tensor_scalar_add(qhatT[:, hp, :], ps, u2[:])
                nc.vector.tensor_scalar_add(qtldT[:, hp, :], ps, v2[:])

            # k^T per head pair
            def emit_kt(kt_hps):
              for hp in kt_hps:
                ps = mps.tile([128, 1024], dt.float32, tag="m")
                for mh in range(2):
                    for ch in range(4):
                        nc.tensor.matmul(
                            ps[:, 512 * mh:512 * (mh + 1)],
                            wqkv[:, ch, 512 + 128 * hp:512 + 128 * (hp + 1)],
                            xt[:, ch, 512 * mh:512 * (mh + 1)],
                            start=(ch == 0), stop=(ch == 3))
                copy_ps(kT[:, hp, :], ps[:])

            # V in [m, hd] layout, two m-chunks per PSUM tile
            def emit_v(v_mc0s):
              for mc0 in v_mc0s:
                ps = mps.tile([128, 1024], dt.float32, tag="m")
                for k2 in range(2):
                    mc = mc0 + k2
                    for ch in range(4):
                        nc.tensor.matmul(
                            ps[:, 512 * k2:512 * (k2 + 1)],
                            xt[:, ch, 128 * mc:128 * (mc + 1)],
                            wqkv[:, ch, 1024:1536],
                            start=(ch == 0), stop=(ch == 3))
                copy_ps(vv[:, mc0:mc0 + 2, :], ps[:])

            emit_kt((0, 1))
            # (kT hp 2-3 and V are woven into the first pipeline slots to
            # keep the PE dense during the first DMA round trips.)

            # ---------------- phase 2: attention, software-pipelined over
            # head PAIRS g=0..3 with stage skew: S1(g) | S2(g-1) | S3+S4+S5
            # (g-2).  Keeps every engine fed while round trips fly.
            lib_inst = nc.gpsimd.load_library(library_config.mlp)

            bdds, acs, attns, attnTs = {}, {}, {}, {}

            def s1_pair(g):                  # bd matmuls -> raw bd -> DRAM
                for h in (2 * g, 2 * g + 1):
                    hp, par = h // 2, h % 2
                    pb = 64 * par
                    bdd = bddp.tile([SEQ, RSTRIDE], dt.bfloat16, tag="bdd")
                    bdds[h] = bdd
                    bdt = hsb.tile([128, 2, RSTRIDE], dt.bfloat16, tag="bdt")
                    nc.vector.memset(bdt[:, :, TOT:RSTRIDE], PADVAL)
                    for n2 in range(2):
                        ps = mps.tile([128, 1024], dt.float32, tag="m")
                        for rh in range(2):
                            nc.tensor.matmul(
                                ps[:, 512 * rh:512 * (rh + 1)],
                                qtldT[pb:pb + 64, hp, 128 * n2:128 * (n2 + 1)],
                                rwt[pb:pb + 64, hp, 512 * rh:512 * (rh + 1)],
                                start=True, stop=True)
                        copy_ps(bdt[:, n2, 0:TOT], ps[:])
                    dst = bass.AP(bdd.tensor, bdd.offset,
                                  [[RSTRIDE, 128], [128 * RSTRIDE, 2],
                                   [1, RSTRIDE]])
                    nc.sync.dma_start(dst, bdt[:])

            def s2_pair(g):                  # ac matmuls; readback accum-adds
                for h in (2 * g, 2 * g + 1):
                    hp, par = h // 2, h % 2
                    pb = 64 * par
                    act_t = ebp.tile([128, 2, TOT], dt.bfloat16, tag="acs")
                    acs[h] = act_t
                    for n2 in range(2):
                        ps = mps.tile([128, 1024], dt.float32, tag="m")
                        for mh in range(2):
                            nc.tensor.matmul(
                                ps[:, 512 * mh:512 * (mh + 1)],
                                qhatT[pb:pb + 64, hp, 128 * n2:128 * (n2 + 1)],
                                kT[pb:pb + 64, hp, 512 * mh:512 * (mh + 1)],
                                start=True, stop=True)
                        copy_ps(act_t[:, n2, :], ps[:])
                    src = bass.AP(
                        bdds[h].tensor, bdds[h].offset + 255,
                        [[RSTRIDE - 1, 128], [128 * (RSTRIDE - 1), 2],
                         [1, TOT]])
                    nc.gpsimd.dma_start(act_t[:], src, accum_op=ALU.add)

            def s3s4_pair(g):                # exp+rowsum, normalize, gather
                for h in (2 * g, 2 * g + 1):
                    attn = hsb.tile([128, 2, TOT], dt.bfloat16, tag="attn")
                    attns[h] = attn
                    for n2 in range(2):
                        rs = hsb.tile([128, 1], dt.float32, tag="rs")
                        au = hsb.tile([128, TOT], dt.bfloat16, tag="au")
                        nc.scalar.activation(au[:], acs[h][:, n2, :],
                                             func=AF.Exp, scale=SCALE,
                                             accum_out=rs[:])
                        rec = hsb.tile([128, 1], dt.float32, tag="rec")
                        nc.vector.reciprocal(rec[:], rs[:])
                        nc.vector.tensor_scalar_mul(attn[:, n2, :], au[:],
                                                    rec[:])
                    attnT = hsb.tile([128, 8, SEQ], dt.bfloat16, tag="attnT")
                    attnTs[h] = attnT
                    gth = nc.gpsimd.dma_gather(
                        out_ap=attnT[:], in_ap=attn[:], idxs_ap=gidx[:],
                        num_idxs=SEQ, num_idxs_reg=SEQ, elem_size=TOT,
                        transpose=True, sbuf_tokens_per_rank=128,
                        sbuf_free_dim_per_rank=2 * TOT,
                        sbuf_free_dim_pad_per_rank=0, sbuf_byte_offset=0)
                    add_dep_helper(gth.ins, lib_inst.ins,
                                   reason="dma_gather needs mlp gpsimd library")

            def s5_pair(g):                  # PV for the pair (col-tiled)
                h0 = 2 * g
                hp = g
                pvt = pvps.tile([128, SEQ], dt.float32, tag="pv")
                for par in range(2):
                    h = h0 + par
                    pb = 64 * par
                    for mc in range(8):
                        nc.tensor.matmul(
                            pvt[pb:pb + 64, :],
                            vv[:, mc, 64 * h:64 * (h + 1)],
                            attnTs[h][:, mc, :],
                            start=(mc == 0), stop=(mc == 7),
                            tile_position=(0, pb))
                    nc.vector.tensor_copy(avt[pb:pb + 64, hp, :],
                                          pvt[pb:pb + 64, :])

            for slot in range(6):
                if slot < 4:
                    s1_pair(slot)
                if slot == 0:
                    emit_v((0, 2))
                    emit_kt((2,))
                if slot == 1:
                    emit_kt((3,))
                    emit_v((4,))
                if 1 <= slot < 5:
                    s2_pair(slot - 1)
                if slot == 1:
                    emit_v((6,))
                if slot >= 2:
                    s3s4_pair(slot - 2)
                    s5_pair(slot - 2)

            # ---------------- phase 3: output projection
            for n2 in range(2):
                psw = mps.tile([128, 1024], dt.float32, tag="m")
                ps = psw[:, 0:512]
                for c4 in range(4):
                    nc.tensor.matmul(ps,
                                     avt[:, c4, 128 * n2:128 * (n2 + 1)],
                                     wout[:, c4, :],
                                     start=(c4 == 0), stop=(c4 == 3))
                osb = hsb.tile([128, 512], dt.float32, tag="osb")
                nc.vector.tensor_add(osb[:], ps, bout[:])
                nc.sync.dma_start(out_d.ap()[128 * n2:128 * (n2 + 1), :], osb[:])


# ---------------------------------------------------------------- host wrapper
_PROGRAM = None


def _get_program():
    global _PROGRAM
    if _PROGRAM is None:
        _PROGRAM = build_program()
    return _PROGRAM


def make_in_maps(x, memory, W_qkv, W_rel, W_out, b_out, u_emb, v_emb):
    x = np.asarray(x, dtype=F32)
    memory = np.asarray(memory, dtype=F32)
    W_qkv = np.asarray(W_qkv, dtype=F32)
    W_rel = np.asarray(W_rel, dtype=F32)
    W_out = np.asarray(W_out, dtype=F32)
    b_out = np.asarray(b_out, dtype=F32)
    u_emb = np.asarray(u_emb, dtype=F32)
    v_emb = np.asarray(v_emb, dtype=F32)

    R = _positional_encoding()                       # (1024, 22)
    RW = R @ W_rel                                   # (1024, 512) = (r, hd)
    rwrev = np.ascontiguousarray(RW[::-1].T)         # (hd, r') = RW[1023-r']
    rwt = _chunked(rwrev, 4).astype(BF16)            # (128, 4, 1024)

    wqkv = _chunked(W_qkv, 4).astype(BF16)           # (128, 4, 1536)
    wout = _chunked(W_out, 4).astype(BF16)           # (128, 4, 512)
    bout = np.tile(b_out[None, :], (128, 1)).astype(F32)
    u2 = np.tile(u_emb, 2)[:, None].astype(F32)
    v2 = np.tile(v_emb, 2)[:, None].astype(F32)
    p = np.arange(128)[:, None] % 16
    s = np.arange(16)[None, :]
    gidx = (s * 16 + p).astype(np.int16)             # (128, 16)

    shared = dict(wqkv=wqkv, rwt=rwt, wout=wout, bout=bout,
                  u2=u2, v2=v2, gidx=gidx)
    in_maps = []
    for c in range(B):
        X = np.concatenate([memory[c], x[c]], axis=0)          # (1024, 512)
        xt = _chunked(np.ascontiguousarray(X.T), 4).astype(BF16)  # (128,4,1024)
        in_maps.append(dict(xt=xt, **shared))
    return in_maps


def run(in_maps, trace=False, **kw):
    nc = _get_program()
    res = run_bass_kernel_spmd(nc, in_maps, core_ids=list(range(B)),
                               trace=trace, **kw)
    out = np.stack([res.results[c]["out"] for c in range(B)]).astype(F32)
    return out, res


def kernel(x, memory, W_qkv, W_rel, W_out, b_out, u_emb, v_emb):
    in_maps = make_in_maps(x, memory, W_qkv, W_rel, W_out, b_out, u_emb, v_emb)
    out, _ = run(in_maps)
    return out.reshape(B, SEQ, DIM)


# revision 20
# speedup vs baseline: 1.0495x; 1.0495x over previous
"""Trainium2 Bass kernel for nn_Attention_386547057357 (Transformer-XL style
relative-position sparse attention).

Sharding: data-parallel over batch — core c computes batch element c.
All weights replicated per core.

Math (per batch element):
    X = [memory; x]  (1024, 512)
    q = x @ W_q  (256, 512);  k = X @ W_k;  v = X @ W_v
    qhat = q + u_emb (per head);  qtld = q + v_emb
    RW = R @ W_rel  (1024, 512)     [host-precomputed constant]
    ac[n, m]  = qhat_h[n] . k_h[m]            (= term_a + term_c)
    bd[n, r'] = qtld_h[n] . RW[1023 - r']     (= term_b + term_d, reversed r)
    scores[n, m] = (ac[n, m] + bd[n, 255 - n + m]) * scale  + causal mask
    out = softmax(scores) @ v @ W_out + b_out

Implementation notes (ADD-form):
  - the relative shift is a DRAM round trip in bf16: write RAW bd rows with
    stride 1281 (cols [1024:1281] = -inf pad -> exp gives 0, the causal
    mask), read back diagonally [[1280, 128], [1, 1024]] at offset 255 with
    DMA accum_op=add onto a tile already holding RAW ac.
  - single ACT exp (scale) bf16-out with fused accum_out rowsum.
  - normalize = DVE tensor_scalar (bf16 tensor x f32 per-partition scalar).
  - attn transpose for the PV matmul via SBUF-source dma_gather(transpose).
  - all PE operands bf16; PSUM accumulation fp32; output fp32.
"""

import sys

sys.path.insert(0, "/opt/trn_rl_repo")

import numpy as np
import ml_dtypes

import concourse.bass as bass
import concourse.mybir as mybir
import concourse.tile as tile
from concourse import bacc, library_config
from concourse.bass_utils import run_bass_kernel_spmd
from concourse.tile_rust import add_dep_helper

BF16 = ml_dtypes.bfloat16
F32 = np.float32

DIM = 512
NHEAD = 8
DHEAD = 64
CTX = 1024
NOCT = 11
B = 8
SEQ = 256
MEM = 768
TOT = MEM + SEQ  # 1024
SCALE = DHEAD ** -0.5  # 0.125
RSTRIDE = 1281  # bd scratch row stride (1024 data + 257 pad)
PADVAL = -120.0  # pad logit: exp(SCALE * (ac - 120)) ~ e^-15 ~ 0 (causal mask)

dt = mybir.dt
AF = mybir.ActivationFunctionType
ALU = mybir.AluOpType


# ---------------------------------------------------------------- host consts
def _positional_encoding():
    coords = np.arange(CTX, dtype=F32)[:, None]
    octaves = np.arange(1 - NOCT, 1, dtype=F32)
    mult = ((2.0 ** octaves) * np.pi).astype(F32)
    scaled = (coords * mult[None, :]).astype(F32)
    return np.concatenate([np.sin(scaled), np.cos(scaled)], axis=-1).astype(F32)


def _chunked(w, nchunk):
    """(128*nchunk, F) -> (128, nchunk, F) with [p, c, f] = w[128c + p, f]."""
    f = w.shape[1]
    return np.ascontiguousarray(w.reshape(nchunk, 128, f).transpose(1, 0, 2))


# ---------------------------------------------------------------- bass program
def build_program():
    nc = bacc.Bacc("TRN2", target_bir_lowering=False, debug=False)

    xt_d = nc.dram_tensor("xt", [128, 4, TOT], dt.bfloat16, kind="ExternalInput")
    wqkv_d = nc.dram_tensor("wqkv", [128, 4, 1536], dt.bfloat16, kind="ExternalInput")
    rwt_d = nc.dram_tensor("rwt", [128, 4, CTX], dt.bfloat16, kind="ExternalInput")
    wout_d = nc.dram_tensor("wout", [128, 4, 512], dt.bfloat16, kind="ExternalInput")
    bout_d = nc.dram_tensor("bout", [128, 512], dt.float32, kind="ExternalInput")
    u2_d = nc.dram_tensor("u2", [128, 1], dt.float32, kind="ExternalInput")
    v2_d = nc.dram_tensor("v2", [128, 1], dt.float32, kind="ExternalInput")
    gidx_d = nc.dram_tensor("gidx", [128, 16], dt.int16, kind="ExternalInput")
    out_d = nc.dram_tensor("out", [SEQ, 512], dt.float32, kind="ExternalOutput")

    with tile.TileContext(nc) as tc:
        _body(tc, xt_d, wqkv_d, rwt_d, wout_d, bout_d, u2_d, v2_d,
              gidx_d, out_d)
    nc.compile()
    return nc


def _body(tc, xt_d, wqkv_d, rwt_d, wout_d, bout_d, u2_d, v2_d, gidx_d, out_d):
    nc = tc.nc
    from contextlib import ExitStack

    with ExitStack() as ctx:
        consts = ctx.enter_context(tc.tile_pool(name="consts", bufs=1))

        # ---- load constants / weights.  xt+wqkv split per chunk so the
        # first projection matmuls can start before the tail of the loads.
        xt = consts.tile([128, 4, TOT], dt.bfloat16)
        wqkv = consts.tile([128, 4, 1536], dt.bfloat16)
        for ch in range(2):
            nc.sync.dma_start(xt[:, ch, :], xt_d.ap()[:, ch, :])
            nc.sync.dma_start(wqkv[:, ch, :], wqkv_d.ap()[:, ch, :])
        u2 = consts.tile([128, 1], dt.float32)
        nc.sync.dma_start(u2[:], u2_d.ap())
        v2 = consts.tile([128, 1], dt.float32)
        nc.sync.dma_start(v2[:], v2_d.ap())
        gidx = consts.tile([128, 16], dt.int16)
        nc.sync.dma_start(gidx[:], gidx_d.ap())
        for ch in range(2, 4):
            nc.sync.dma_start(xt[:, ch, :], xt_d.ap()[:, ch, :])
            nc.sync.dma_start(wqkv[:, ch, :], wqkv_d.ap()[:, ch, :])
        rwt = consts.tile([128, 4, CTX], dt.bfloat16)
        nc.sync.dma_start(rwt[:], rwt_d.ap())
        wout = consts.tile([128, 4, 512], dt.bfloat16)
        nc.sync.dma_start(wout[:], wout_d.ap())
        bout = consts.tile([128, 512], dt.float32)
        nc.sync.dma_start(bout[:], bout_d.ap())

        # persistent intermediates
        qhatT = consts.tile([128, 4, SEQ], dt.bfloat16)  # (q+u)^T  [hd, n]
        qtldT = consts.tile([128, 4, SEQ], dt.bfloat16)  # (q+v)^T  [hd, n]
        kT = consts.tile([128, 4, TOT], dt.bfloat16)     # k^T      [hd, m]
        vv = consts.tile([128, 8, 512], dt.bfloat16)     # V        [m, hd]
        avt = consts.tile([128, 4, SEQ], dt.bfloat16)    # attnV^T  [hd, n]

        with (
            tc.tile_pool(name="mps", bufs=3, space="PSUM") as mps,
            tc.tile_pool(name="pvps", bufs=2, space="PSUM") as pvps,
            tc.tile_pool(name="hsb", bufs=4) as hsb,
            tc.tile_pool(name="bdt_p", bufs=6) as bdtp,
            tc.tile_pool(name="ebp", bufs=8) as ebp,
            tc.tile_pool(name="bdd", bufs=8, space="DRAM") as bddp,
        ):
            # alternate PSUM->SBUF cast copies between ACT and DVE to
            # balance the two engines
            _cp = [0]

            def copy_ps(dst, src):
                _cp[0] ^= 1
                if _cp[0]:
                    nc.scalar.copy(dst, src)
                else:
                    nc.vector.tensor_copy(dst, src)

            # q^T per head pair, then add u/v
            for hp in range(4):
                psw = mps.tile([128, 1024], dt.float32, tag="m")
                ps = psw[:, 0:SEQ]
                for ch in range(4):
                    nc.tensor.matmul(ps, wqkv[:, ch, 128 * hp:128 * (hp + 1)],
                                     xt[:, ch, MEM:TOT],
                                     start=(ch == 0), stop=(ch == 3))
                # tensor_scalar with f32 psum in, bf16 out (probe-verified)
                nc.vector.tensor_scalar_add(qhatT[:, hp, :], ps, u2[:])
                nc.vector.tensor_scalar_add(qtldT[:, hp, :], ps, v2[:])

            # k^T per head pair
            def emit_kt(kt_hps):
              for hp in kt_hps:
                ps = mps.tile([128, 1024], dt.float32, tag="m")
                for mh in range(2):
                    for ch in range(4):
                        nc.tensor.matmul(
                            ps[:, 512 * mh:512 * (mh + 1)],
                            wqkv[:, ch, 512 + 128 * hp:512 + 128 * (hp + 1)],
                            xt[:, ch, 512 * mh:512 * (mh + 1)],
                            start=(ch == 0), stop=(ch == 3))
                copy_ps(kT[:, hp, :], ps[:])

            # V in [m, hd] layout, two m-chunks per PSUM tile
            def emit_v(v_mc0s):
              for mc0 in v_mc0s:
                ps = mps.tile([128, 1024], dt.float32, tag="m")
                for k2 in range(2):
                    mc = mc0 + k2
                    for ch in range(4):
                        nc.tensor.matmul(
                            ps[:, 512 * k2:512 * (k2 + 1)],
                            xt[:, ch, 128 * mc:128 * (mc + 1)],
                            wqkv[:, ch, 1024:1536],
                            start=(ch == 0), stop=(ch == 3))
                copy_ps(vv[:, mc0:mc0 + 2, :], ps[:])

            emit_kt((0, 1))
            # (kT hp 2-3 and V are woven into the first pipeline slots to
            # keep the PE dense during the first DMA round trips.)

            # ---------------- phase 2: attention, software-pipelined over
            # head PAIRS g=0..3 with stage skew: S1(g) | S2(g-1) | S3+S4+S5
            # (g-2).  Keeps every engine fed while round trips fly.
            lib_inst = nc.gpsimd.load_library(library_config.mlp)

            bdds, acs, attns, attnTs = {}, {}, {}, {}

            def s1_pair(g):                  # bd matmuls -> raw bd -> DRAM
                for h in (2 * g, 2 * g + 1):
                    hp, par = h // 2, h % 2
                    pb = 64 * par
                    bdd = bddp.tile([SEQ, RSTRIDE], dt.bfloat16, tag="bdd")
                    bdds[h] = bdd
                    bdt = bdtp.tile([128, 2, RSTRIDE], dt.bfloat16, tag="bdt")
                    nc.vector.memset(bdt[:, :, TOT:RSTRIDE], PADVAL)
                    for n2 in range(2):
                        ps = mps.tile([128, 1024], dt.float32, tag="m")
                        for rh in range(2):
                            nc.tensor.matmul(
                                ps[:, 512 * rh:512 * (rh + 1)],
                                qtldT[pb:pb + 64, hp, 128 * n2:128 * (n2 + 1)],
                                rwt[pb:pb + 64, hp, 512 * rh:512 * (rh + 1)],
                                start=True, stop=True)
                        copy_ps(bdt[:, n2, 0:TOT], ps[:])
                    dst = bass.AP(bdd.tensor, bdd.offset,
                                  [[RSTRIDE, 128], [128 * RSTRIDE, 2],
                                   [1, RSTRIDE]])
                    nc.sync.dma_start(dst, bdt[:])

            def s2_pair(g):                  # ac matmuls; readback accum-adds
                for h in (2 * g, 2 * g + 1):
                    hp, par = h // 2, h % 2
                    pb = 64 * par
                    act_t = ebp.tile([128, 2, TOT], dt.bfloat16, tag="acs")
                    acs[h] = act_t
                    for n2 in range(2):
                        ps = mps.tile([128, 1024], dt.float32, tag="m")
                        for mh in range(2):
                            nc.tensor.matmul(
                                ps[:, 512 * mh:512 * (mh + 1)],
                                qhatT[pb:pb + 64, hp, 128 * n2:128 * (n2 + 1)],
                                kT[pb:pb + 64, hp, 512 * mh:512 * (mh + 1)],
                                start=True, stop=True)
                        copy_ps(act_t[:, n2, :], ps[:])
                    src = bass.AP(
                        bdds[h].tensor, bdds[h].offset + 255,
                        [[RSTRIDE - 1, 128], [128 * (RSTRIDE - 1), 2],
                         [1, TOT]])
                    nc.gpsimd.dma_start(act_t[:], src, accum_op=ALU.add)

            def s3s4_pair(g):                # exp+rowsum, normalize, gather
                for h in (2 * g, 2 * g + 1):
                    attn = hsb.tile([128, 2, TOT], dt.bfloat16, tag="attn")
                    attns[h] = attn
                    for n2 in range(2):
                        rs = hsb.tile([128, 1], dt.float32, tag="rs")
                        au = hsb.tile([128, TOT], dt.bfloat16, tag="au")
                        nc.scalar.activation(au[:], acs[h][:, n2, :],
                                             func=AF.Exp, scale=SCALE,
                                             accum_out=rs[:])
                        rec = hsb.tile([128, 1], dt.float32, tag="rec")
                        nc.vector.reciprocal(rec[:], rs[:])
                        nc.vector.tensor_scalar_mul(attn[:, n2, :], au[:],
                                                    rec[:])
                    attnT = hsb.tile([128, 8, SEQ], dt.bfloat16, tag="attnT")
                    attnTs[h] = attnT
                    gth = nc.gpsimd.dma_gather(
                        out_ap=attnT[:], in_ap=attn[:], idxs_ap=gidx[:],
                        num_idxs=SEQ, num_idxs_reg=SEQ, elem_size=TOT,
                        transpose=True, sbuf_tokens_per_rank=128,
                        sbuf_free_dim_per_rank=2 * TOT,
                        sbuf_free_dim_pad_per_rank=0, sbuf_byte_offset=0)
                    add_dep_helper(gth.ins, lib_inst.ins,
                                   reason="dma_gather needs mlp gpsimd library")

            def s5_pair(g):                  # PV for the pair (col-tiled)
                h0 = 2 * g
                hp = g
                pvt = pvps.tile([128, SEQ], dt.float32, tag="pv")
                for par in range(2):
                    h = h0 + par
                    pb = 64 * par
                    for mc in range(8):
                        nc.tensor.matmul(
                            pvt[pb:pb + 64, :],
                            vv[:, mc, 64 * h:64 * (h + 1)],
                            attnTs[h][:, mc, :],
                            start=(mc == 0), stop=(mc == 7),
                            tile_position=(0, pb))
                    nc.vector.tensor_copy(avt[pb:pb + 64, hp, :],
                                          pvt[pb:pb + 64, :])

            # super-stage schedule: group 1's round trips fly while group 0's
            # exp/gather/PV phase runs (and vice-versa fill for group 0 comes
            # from the remaining projection matmuls).
            emit_v((0, 2))
            s1_pair(0); s1_pair(1)           # bd g0 + write DMAs
            s2_pair(0); s2_pair(1)           # ac g0 + accum readbacks
            emit_kt((2, 3))                  # fill g0's round-trip window
            s1_pair(2); s1_pair(3)           # bd g1 + writes
            emit_v((4, 6))
            s2_pair(2); s2_pair(3)           # ac g1 + accums
            s3s4_pair(0); s3s4_pair(1)       # exp/norm/gather g0
            s5_pair(0); s5_pair(1)           # PV g0
            s3s4_pair(2); s3s4_pair(3)       # exp/norm/gather g1
            s5_pair(2); s5_pair(3)           # PV g1

            # ---------------- phase 3: output projection
            for n2 in range(2):
                psw = mps.tile([128, 1024], dt.float32, tag="m")
                ps = psw[:, 0:512]
                for c4 in range(4):
                    nc.tensor.matmul(ps,
                                     avt[:, c4, 128 * n2:128 * (n2 + 1)],
                                     wout[:, c4, :],
                                     start=(c4 == 0), stop=(c4 == 3))
                osb = hsb.tile([128, 512], dt.float32, tag="osb")
                nc.vector.tensor_add(osb[:], ps, bout[:])
                nc.sync.dma_start(out_d.ap()[128 * n2:128 * (n2 + 1), :], osb[:])


# ---------------------------------------------------------------- host wrapper
_PROGRAM = None


def _get_program():
    global _PROGRAM
    if _PROGRAM is None:
        _PROGRAM = build_program()
    return _PROGRAM


def make_in_maps(x, memory, W_qkv, W_rel, W_out, b_out, u_emb, v_emb):
    x = np.asarray(x, dtype=F32)
    memory = np.asarray(memory, dtype=F32)
    W_qkv = np.asarray(W_qkv, dtype=F32)
    W_rel = np.asarray(W_rel, dtype=F32)
    W_out = np.asarray(W_out, dtype=F32)
    b_out = np.asarray(b_out, dtype=F32)
    u_emb = np.asarray(u_emb, dtype=F32)
    v_emb = np.asarray(v_emb, dtype=F32)

    R = _positional_encoding()                       # (1024, 22)
    RW = R @ W_rel                                   # (1024, 512) = (r, hd)
    rwrev = np.ascontiguousarray(RW[::-1].T)         # (hd, r') = RW[1023-r']
    rwt = _chunked(rwrev, 4).astype(BF16)            # (128, 4, 1024)

    wqkv = _chunked(W_qkv, 4).astype(BF16)           # (128, 4, 1536)
    wout = _chunked(W_out, 4).astype(BF16)           # (128, 4, 512)
    bout = np.tile(b_out[None, :], (128, 1)).astype(F32)
    u2 = np.tile(u_emb, 2)[:, None].astype(F32)
    v2 = np.tile(v_emb, 2)[:, None].astype(F32)
    p = np.arange(128)[:, None] % 16
    s = np.arange(16)[None, :]
    gidx = (s * 16 + p).astype(np.int16)             # (128, 16)

    shared = dict(wqkv=wqkv, rwt=rwt, wout=wout, bout=bout,
                  u2=u2, v2=v2, gidx=gidx)
    in_maps = []
    for c in range(B):
        X = np.concatenate([memory[c], x[c]], axis=0)          # (1024, 512)
        xt = _chunked(np.ascontiguousarray(X.T), 4).astype(BF16)  # (128,4,1024)
        in_maps.append(dict(xt=xt, **shared))
    return in_maps


def run(in_maps, trace=False, **kw):
    nc = _get_program()
    res = run_bass_kernel_spmd(nc, in_maps, core_ids=list(range(B)),
                               trace=trace, **kw)
    out = np.stack([res.results[c]["out"] for c in range(B)]).astype(F32)
    return out, res


def kernel(x, memory, W_qkv, W_rel, W_out, b_out, u_emb, v_emb):
    in_maps = make_in_maps(x, memory, W_qkv, W_rel, W_out, b_out, u_emb, v_emb)
    out, _ = run(in_maps)
    return out.reshape(B, SEQ, DIM)
